# revision 1
# baseline (speedup 1.0000x reference)
"""CondConv (per-sample dynamic conv) Trainium2 Bass kernel.

Reference computation (per sample b):
    gap     = mean(x[b], spatial)                    # [C]
    r       = sigmoid(fc_w @ gap + fc_b)             # [E]
    comb    = sum_e r[e] * kernel_weights[e]         # [O, I, 3, 3]
    y[b]    = conv2d(x[b], comb, pad=1)              # [O, H, W]

Sharding: data-parallel over batch, 4 samples per core on 8 cores.
Expert kernels + fc params replicated to every core.

Per-core dataflow (v2 — startup-latency optimized):
  - All HBM loads ride the two HWDGE rings (sync + scalar), ordered by
    need: sample 0's x chunks first, then the expert-weight (oh0,ci0)
    block split across both rings, the other three W blocks, then x of
    samples 1-3.  y stores ride the sync ring afterward.
  - W host layout is [oh][ci][tap][e][oin] so each (oh,ci) block is one
    contiguous big-packet DMA and synthesis reads per-expert strided
    views.
  - Sample 0's staging is split into half-chunks per ci: ci0 cast+GAP
    on ScalarE (activation accum), ci1 on VectorE (tensor_copy +
    reduce_sum), so routing completes ~11us instead of ~38us.
  - Synthesis on DVE: per (oh,ci) chunk, tensor_scalar(4x bf16) +
    tensor_tensor add (2x bf16) over the expert stack.
  - Conv per (s,oh) block: tap-outer / tile-inner over 7 PSUM banks:
    for ci,tap: one lhsT slice streams 7 matmuls (N=448), so each PSUM
    tile accumulates 18 matmuls; ScalarE evacuates to a per-(s,oh)
    SBUF tile; quarter-stores DMA out as rows complete.
"""

import numpy as np
import ml_dtypes

B, C, H, W = 32, 256, 56, 56
E = 8
N_CORES = 8
BL = B // N_CORES          # local batch per core
HP = 58                    # padded rows (1 top + 1 bottom)
WP = 60                    # padded cols (2 left + 2 right: keeps the
                           # bf16 interior 4B-aligned for DVE/ACT 2x)
HWP = HP * WP              # 3480
HWU = H * W                # 3136 (unpadded)
HH = H // 2                # 28 rows per staging half-chunk
TAPS = 9
OIN = 128                  # output channels per half
CIBLK = TAPS * OIN         # per (oh, ci) combined block = 1152
OHBLK = 2 * CIBLK          # per (oh) block = 2304
EBLK = 2 * OHBLK           # combined weights per sample = 4608
WBLK = TAPS * E * OIN      # per (oh, ci) expert-stack block = 9216
ROWS = 8                   # output rows per n-tile
NT = H // ROWS             # 7 n-tiles
NF = ROWS * W              # 448 matmul free dim

_CACHE = {}


def _build():
    import concourse.bacc as bacc
    import concourse.mybir as mybir
    import concourse.tile as tile
    from contextlib import ExitStack

    dt = mybir.dt
    AF = mybir.ActivationFunctionType
    Alu = mybir.AluOpType
    AX = mybir.AxisListType

    nc = bacc.Bacc(
        "TRN2",
        target_bir_lowering=False,
        debug=False,
        enable_asserts=False,
        num_devices=N_CORES,
    )
    x_d = nc.dram_tensor("x", [BL, C, H, W], dt.float32, kind="ExternalInput")
    # host layout per partition p (= i % 128): [oh, ci, tap, e, oin]
    w_d = nc.dram_tensor("wp", [128, 4 * WBLK], dt.bfloat16, kind="ExternalInput")
    fcw_d = nc.dram_tensor("fcw", [C, E], dt.float32, kind="ExternalInput")
    fcb_d = nc.dram_tensor("fcb", [E, 1], dt.float32, kind="ExternalInput")
    eye_d = nc.dram_tensor("eye", [E, E], dt.float32, kind="ExternalInput")
    y_d = nc.dram_tensor("y", [BL, C, H, W], dt.float32, kind="ExternalOutput")

    with tile.TileContext(nc) as tc:
        with ExitStack() as ctx:
            cpool = ctx.enter_context(tc.tile_pool(name="consts", bufs=1))
            stgpool = ctx.enter_context(tc.tile_pool(name="stg", bufs=1))
            xpool = ctx.enter_context(tc.tile_pool(name="xs", bufs=3))
            combpool = ctx.enter_context(tc.tile_pool(name="combs", bufs=2))
            spool = ctx.enter_context(tc.tile_pool(name="small", bufs=2))
            opool = ctx.enter_context(tc.tile_pool(name="outs", bufs=2))
            pspool = ctx.enter_context(tc.tile_pool(name="cpsum", bufs=1, space="PSUM"))
            psmall = ctx.enter_context(tc.tile_pool(name="spsum", bufs=1, space="PSUM"))

            w_sb = cpool.tile([128, 4 * WBLK], dt.bfloat16)
            wv = w_sb.rearrange(
                "p (oh ci e tap o) -> p oh ci e tap o", oh=2, ci=2, e=E, tap=TAPS, o=OIN
            )
            fcw_sb = cpool.tile([128, 2 * E], dt.float32)
            fcb_sb = cpool.tile([E, 1], dt.float32)
            eye_sb = cpool.tile([E, E], dt.float32)

            xvs, gaps, rbs, combs = {}, {}, {}, {}

            # ---- DMA emission helpers (explicit ring assignment) ----
            def dma_w(eng, oh, ci, e0, e1):
                base = (oh * 2 + ci) * WBLK + e0 * CIBLK
                hi = (oh * 2 + ci) * WBLK + e1 * CIBLK
                eng.dma_start(out=w_sb[:, base:hi], in_=w_d.ap()[:, base:hi])

            def dma_consts_fcw():
                for ci in range(2):
                    nc.scalar.dma_start(
                        out=fcw_sb[:, ci * E : (ci + 1) * E],
                        in_=fcw_d.ap()[ci * 128 : (ci + 1) * 128, :],
                    )
                nc.scalar.dma_start(out=fcb_sb[:], in_=fcb_d.ap())

            # ---- staging: x DMA + cast to padded bf16 + GAP ----
            def dma_x_chunk(eng, s, ci, h0, h1, tag):
                xg = stgpool.tile([128, (h1 - h0) * W], dt.float32, tag=tag)
                eng.dma_start(
                    out=xg[:],
                    in_=x_d.ap()[s, ci * 128 : (ci + 1) * 128, h0:h1, :],
                )
                return xg

            xts = {}

            def make_xt(s):
                xt = xpool.tile([128, 2 * HWP], dt.bfloat16, tag="xt")
                xv = xt.rearrange("p (c h w) -> p c h w", c=2, h=HP, w=WP)
                xvs[s] = xv
                xts[s] = xt
                for ci in range(2):
                    nc.vector.memset(xv[:, ci, 0, :], 0.0)
                    nc.vector.memset(xv[:, ci, HP - 1, :], 0.0)
                    nc.vector.memset(xv[:, ci, 1 : HP - 1, 0:2], 0.0)
                    nc.vector.memset(xv[:, ci, 1 : HP - 1, WP - 2 : WP], 0.0)
                g = spool.tile([128, 4], dt.float32, tag="gap")
                gaps[s] = g
                return xv, g

            def cast_chunk_act(s, ci, h0, h1, xg, gcol):
                # fp32 -> bf16 into the padded layout AND spatial-sum for GAP
                xv = xvs[s]
                xgv = xg.rearrange("p (h w) -> p h w", h=h1 - h0, w=W)
                nc.scalar.activation(
                    out=xv[:, ci, 1 + h0 : 1 + h1, 2 : 2 + W],
                    in_=xgv[:],
                    func=AF.Copy,
                    accum_out=gaps[s][:, gcol : gcol + 1],
                )

            def cast_chunk_dve(s, ci, h0, h1, xg, gcol):
                xv = xvs[s]
                xgv = xg.rearrange("p (h w) -> p h w", h=h1 - h0, w=W)
                nc.vector.tensor_copy(
                    out=xv[:, ci, 1 + h0 : 1 + h1, 2 : 2 + W], in_=xgv[:]
                )
                nc.vector.reduce_sum(
                    gaps[s][:, gcol : gcol + 1], xg[:], axis=AX.X
                )

            def stage_dma(s):
                xga = dma_x_chunk(nc.sync, s, 0, 0, HH, "xg0a")
                xgb = dma_x_chunk(nc.sync, s, 0, HH, H, "xg0b")
                xgc = dma_x_chunk(nc.scalar, s, 1, 0, HH, "xg1a")
                xgd = dma_x_chunk(nc.scalar, s, 1, HH, H, "xg1b")
                return xga, xgb, xgc, xgd

            def stage_cast(s, xgs):
                xga, xgb, xgc, xgd = xgs
                cast_chunk_act(s, 0, 0, HH, xga, 0)
                cast_chunk_act(s, 0, HH, H, xgb, 1)
                cast_chunk_act(s, 1, 0, HH, xgc, 2)
                cast_chunk_act(s, 1, HH, H, xgd, 3)

            def stage_steady(s):
                stage_cast(s, stage_dma(s))

            # ---- routing: logits -> sigmoid -> broadcast to 128p ----
            def route(s, gcols):
                g = gaps[s]
                prt = psmall.tile([128, E], dt.float32, tag="prt")
                pl = prt[0:E, 0:1]
                for k, (ci, gcol) in enumerate(gcols):
                    nc.tensor.matmul(
                        pl,
                        lhsT=fcw_sb[:, ci * E : (ci + 1) * E],
                        rhs=g[:, gcol : gcol + 1],
                        start=(k == 0),
                        stop=(k == len(gcols) - 1),
                    )
                rr = spool.tile([E, 1], dt.float32, tag="rr")
                nc.scalar.activation(
                    out=rr[:], in_=pl, func=AF.Sigmoid, bias=fcb_sb[:], scale=1.0
                )
                # broadcast r to all 128 partitions via eye-matmul
                nc.tensor.matmul(
                    prt[:, 0:E],
                    lhsT=rr[:].broadcast_to([E, 128]),
                    rhs=eye_sb[:],
                    start=True,
                    stop=True,
                )
                rb = spool.tile([128, E], dt.float32, tag="rb")
                nc.scalar.activation(out=rb[:], in_=prt[:, 0:E], func=AF.Copy)
                rbs[s] = rb

            # ---- synthesis of one (oh, ci) chunk ----
            # Default: all-DVE chain of tensor_scalar (scale) + tensor_tensor
            # (accumulate).  For latency-critical chunks (sample 0), split by
            # tap range and offload the scale of experts 5-7 to ScalarE so
            # the DVE chain shortens.
            def synth_chunk(s, oh, ci, splits=((0, TAPS),), act_experts=()):
                cb = combs[s]
                rb = rbs[s]
                wbase = (oh * 2 + ci) * WBLK
                cbase = (oh * 2 + ci) * CIBLK
                for t0, t1 in splits:
                    fd = (t1 - t0) * OIN
                    dst = cb[:, cbase + t0 * OIN : cbase + t1 * OIN]
                    atmps = {}
                    for e in act_experts:
                        lo = wbase + e * CIBLK + t0 * OIN
                        at = spool.tile(
                            [128, fd], dt.bfloat16, tag=f"atmp{e}", name=f"atmp{e}"
                        )
                        nc.scalar.activation(
                            out=at[:],
                            in_=w_sb[:, lo : lo + fd],
                            func=AF.Copy,
                            scale=rb[:, e : e + 1],
                        )
                        atmps[e] = at
                    first = True
                    for e in range(E):
                        lo = wbase + e * CIBLK + t0 * OIN
                        src = w_sb[:, lo : lo + fd]
                        if e in atmps:
                            nc.vector.tensor_tensor(
                                out=dst, in0=atmps[e][:], in1=dst, op=Alu.add
                            )
                        elif first:
                            nc.vector.tensor_scalar_mul(dst, src, rb[:, e : e + 1])
                            first = False
                        else:
                            tmp = spool.tile(
                                [128, fd], dt.bfloat16, tag="stmp", name="stmp"
                            )
                            nc.vector.tensor_scalar_mul(tmp[:], src, rb[:, e : e + 1])
                            nc.vector.tensor_tensor(
                                out=dst, in0=tmp[:], in1=dst, op=Alu.add
                            )

            def new_cb(s):
                cb = combpool.tile([128, EBLK], dt.bfloat16, tag="cb")
                combs[s] = cb

            # ---- conv of one (s, oh) block: tap-outer, tile-inner ----
            def conv_block(s, oh, last=False):
                xv = xvs[s]
                cb = combs[s]
                ot = opool.tile([128, HWU], dt.float32, tag="ot")
                pss = [
                    pspool.tile([128, NF], dt.float32, tag=f"ps{nt}", name=f"ps{nt}")
                    for nt in range(NT)
                ]
                k = 0
                for ci in range(2):
                    for kh in range(3):
                        for kw in range(3):
                            tap = kh * 3 + kw
                            lo = (oh * 2 + ci) * CIBLK + tap * OIN
                            lhsT = cb[:, lo : lo + OIN]
                            for nt in range(NT):
                                r0 = nt * ROWS
                                nc.tensor.matmul(
                                    pss[nt],
                                    lhsT=lhsT,
                                    rhs=xv[
                                        :,
                                        ci,
                                        r0 + kh : r0 + kh + ROWS,
                                        kw + 1 : kw + 1 + W,
                                    ],
                                    start=(k == 0),
                                    stop=(k == 17),
                                )
                            k += 1
                otv = ot.rearrange("p (h w) -> p h w", h=H, w=W)
                # evacuate PSUM -> SBUF; store as rows complete, spreading
                # the stores across both HWDGE rings.  The last block uses
                # per-tile stores and ACT/DVE-split evacuation to shrink the
                # kernel tail.
                if last:
                    stores = {nt: (nt * ROWS, (nt + 1) * ROWS) for nt in range(NT)}
                else:
                    stores = {1: (0, 14), 3: (14, 28), 5: (28, 42), 6: (42, 56)}
                half = 0
                for nt in range(NT):
                    dstc = ot[:, nt * NF : (nt + 1) * NF]
                    if last and nt % 2 == 1:
                        nc.vector.tensor_copy(out=dstc, in_=pss[nt][:])
                    else:
                        nc.scalar.activation(out=dstc, in_=pss[nt][:], func=AF.Copy)
                    if nt in stores:
                        qa, qb = stores[nt]
                        eng = nc.sync if half == 0 else nc.scalar
                        half ^= 1
                        eng.dma_start(
                            out=y_d.ap()[s, oh * 128 : (oh + 1) * 128, qa:qb, :],
                            in_=otv[:, qa:qb, :],
                        )

            # ================= emission =================
            # Ring plan:
            #   sync HWDGE:   x ci0 chunks | eye | all W e0-3 halves | half of y
            #   scalar HWDGE: x ci1 chunks | fcw,fcb | other half of y
            #   gpsimd SWDGE: all W e4-7 halves (keeps bulk-W triggers off the
            #                 compute engines; ring-full trigger stalls only
            #                 block the otherwise-idle gpsimd engine)
            # PE warm-up: a dummy matmul batch brings the HAM clock-gate to
            # 8/8 before the conv stream starts.
            warm = cpool.tile([128, 576], dt.bfloat16)
            nc.vector.memset(warm[:], 0.0)
            xv0, g0 = make_xt(0)
            xg00a = dma_x_chunk(nc.sync, 0, 0, 0, HH, "xg0a")
            xg00b = dma_x_chunk(nc.sync, 0, 0, HH, H, "xg0b")
            xg01a = dma_x_chunk(nc.scalar, 0, 1, 0, HH, "xg1a")
            dma_consts_fcw()
            xg01b = dma_x_chunk(nc.scalar, 0, 1, HH, H, "xg1b")
            dma_w(nc.gpsimd, 0, 0, 4, 8)
            nc.sync.dma_start(out=eye_sb[:], in_=eye_d.ap())
            dma_w(nc.sync, 0, 0, 0, 2)
            dma_w(nc.sync, 0, 0, 2, 4)
            dma_w(nc.sync, 0, 1, 0, 4)
            dma_w(nc.sync, 1, 0, 0, 4)
            dma_w(nc.sync, 1, 1, 0, 4)

            def warm_mms(n):
                psw = pspool.tile([128, NF], dt.float32, tag="ps0", name="psw")
                for i in range(n):
                    nc.tensor.matmul(
                        psw[:], lhsT=warm[:, 448:576], rhs=warm[:, 0:448],
                        start=(i == 0), stop=(i == n - 1),
                    )

            cast_chunk_act(0, 0, 0, HH, xg00a, 0)
            cast_chunk_act(0, 0, HH, H, xg00b, 1)
            cast_chunk_dve(0, 1, 0, HH, xg01a, 2)
            cast_chunk_dve(0, 1, HH, H, xg01b, 3)
            route(0, [(0, 0), (0, 1), (1, 2), (1, 3)])
            dma_w(nc.scalar, 0, 1, 4, 8)
            warm_mms(30)

            new_cb(0)
            s0_splits = ((0, 5), (5, TAPS))
            s0_act = (5, 6, 7)
            synth_chunk(0, 0, 0, splits=s0_splits)
            synth_chunk(0, 0, 1, splits=s0_splits, act_experts=s0_act)
            dma_w(nc.scalar, 1, 0, 4, 8)
            dma_w(nc.scalar, 1, 1, 4, 8)
            conv_block(0, 0)

            xv1, g1 = make_xt(1)
            stage_steady(1)
            route(1, [(0, 0), (0, 1), (1, 2), (1, 3)])
            synth_chunk(0, 1, 0, act_experts=s0_act)
            synth_chunk(0, 1, 1, act_experts=s0_act)
            conv_block(0, 1)

            new_cb(1)
            synth_chunk(1, 0, 0, act_experts=s0_act)
            synth_chunk(1, 0, 1, act_experts=s0_act)
            conv_block(1, 0)

            xv2, g2 = make_xt(2)
            stage_steady(2)
            route(2, [(0, 0), (0, 1), (1, 2), (1, 3)])
            synth_chunk(1, 1, 0)
            synth_chunk(1, 1, 1)
            conv_block(1, 1)

            new_cb(2)
            synth_chunk(2, 0, 0)
            synth_chunk(2, 0, 1)
            conv_block(2, 0)

            xv3, g3 = make_xt(3)
            stage_steady(3)
            route(3, [(0, 0), (0, 1), (1, 2), (1, 3)])
            synth_chunk(2, 1, 0)
            synth_chunk(2, 1, 1)
            conv_block(2, 1)

            new_cb(3)
            synth_chunk(3, 0, 0)
            synth_chunk(3, 0, 1)
            conv_block(3, 0)
            synth_chunk(3, 1, 0)
            synth_chunk(3, 1, 1)
            conv_block(3, 1, last=True)

    nc.compile()
    return nc


def _get_nc():
    if "nc" not in _CACHE:
        _CACHE["nc"] = _build()
    return _CACHE["nc"]


def _pack_inputs(x, kernel_weights, fc_w, fc_b):
    # w layout per partition p (= i % 128): [oh, ci, e, tap, oin], bf16
    a = np.asarray(kernel_weights, np.float32).reshape(E, 2, 128, 2, 128, 3, 3)
    # dims: e, oh, oin, ci, p, kh, kw -> p, oh, ci, e, (kh kw), oin
    a = np.ascontiguousarray(a.transpose(4, 1, 3, 0, 5, 6, 2)).reshape(128, 4 * WBLK)
    wp = a.astype(ml_dtypes.bfloat16)
    fcw_t = np.ascontiguousarray(np.asarray(fc_w, np.float32).T / float(H * W))
    fcb2 = np.ascontiguousarray(np.asarray(fc_b, np.float32).reshape(E, 1))
    eye = np.eye(E, dtype=np.float32)
    x = np.ascontiguousarray(np.asarray(x, np.float32))
    in_maps = []
    for i in range(N_CORES):
        in_maps.append(
            {
                "x": x[i * BL : (i + 1) * BL],
                "wp": wp,
                "fcw": fcw_t,
                "fcb": fcb2,
                "eye": eye,
            }
        )
    return in_maps


def _run(x, kernel_weights, fc_w, fc_b, trace=False):
    from concourse.bass_utils import run_bass_kernel_spmd

    nc = _get_nc()
    in_maps = _pack_inputs(x, kernel_weights, fc_w, fc_b)
    res = run_bass_kernel_spmd(nc, in_maps, core_ids=list(range(N_CORES)), trace=trace)
    y = np.concatenate([res.results[i]["y"] for i in range(N_CORES)], axis=0)
    return np.ascontiguousarray(y.astype(np.float32)), res


def kernel(x, kernel_weights, fc_w, fc_b):
    y, _ = _run(x, kernel_weights, fc_w, fc_b, trace=False)
    return y


def kernel_traced(x, kernel_weights, fc_w, fc_b):
    y, res = _run(x, kernel_weights, fc_w, fc_b, trace=True)
    return y, res

